# revision 45
# baseline (speedup 1.0000x reference)
"""Causal self-attention (B=4, T=2048, C=1024, H=16, D=64) on 8 TRN2 NeuronCores.

Sharding: 8 cores = 4 batches x 2 head-groups (8 heads each). Each core:
  - QKV projection for its (batch, head-group) column slice of w_attn,
    producing qT/kT in [d, t] layout and v in [t, d].
  - Causal attention with softmax denominators from 64 replicated
    ones-columns appended to V (no cross-partition reduction needed).
  - Row-sharded output projection -> per-core partial [T, C].
Host sums the two partials per batch and adds b_proj.

Head-pair layout: heads (2m, 2m+1) share one [128, T] qT/kT tile
(partitions 0-63 / 64-127), so the two K=64 QK matmuls of a pair run
concurrently as row-tiles of the PE array (tile_position (0,0)/(64,0)),
writing the two 512-col halves of one [128, 1024] PSUM tile.  A single
exp activation then covers both heads, halving ACT instruction count.
Attention is processed in 512-wide q-chunks; chunk c only needs
projection quarter c of Q (and quarters <= c of K/V), so attention
starts right after quarter 0 and the projection quarters + the output
projection interleave with it to keep the PE busy while ACT runs exp.

Matmul operands are bf16 (1 cycle/row on the PE) with all accumulation
in fp32 PSUM.  Input DMAs are batched into ~10 large descriptors spread
over the SP and ACT HWDGE rings.
"""

import sys
import types

import numpy as np

B, T, C, H, D = 4, 2048, 1024, 16, 64
HG = 8            # heads per core
CG = HG * D       # 512 channels per group
NP = HG // 2      # 4 head-pairs per core
NCORES = 8
TB = T // 128     # 16 t-blocks (also k-blocks)
CT = C // 128     # 8 c-chunks of the contraction dim
NCH = 4           # 512-wide q-chunks


def _register_ntff_hook():
    """Register the axon NTFF profile hook if the image's antenv lacks it."""
    try:
        import antenv
        if getattr(antenv, "axon_hooks", None) is not None:
            return
        from trn_agent_boot.trn_boot import _ntff_profile_via_ctypes
        hook = _ntff_profile_via_ctypes("/opt/axon/libaxon_pjrt.so")
        mod = types.ModuleType("antenv.axon_hooks")
        mod._hook = hook
        mod.get_axon_ntff_profile_hook = lambda: mod._hook
        mod.set_axon_ntff_profile_hook = lambda h: setattr(mod, "_hook", h)
        sys.modules["antenv.axon_hooks"] = mod
        antenv.axon_hooks = mod
    except Exception:
        pass


_NC_CACHE = {}


def _build():
    import concourse.bacc as bacc
    import concourse.mybir as mybir
    import concourse.tile as tile
    from concourse.masks import make_upper_triangular
    from contextlib import ExitStack

    F32 = mybir.dt.float32
    BF16 = mybir.dt.bfloat16
    MUL = mybir.AluOpType.mult
    EXP = mybir.ActivationFunctionType.Exp

    nc = bacc.Bacc(None, target_bir_lowering=False, debug=False)
    # All inputs host-pre-arranged to [128, ...] partition-major layouts so
    # every DMA is a contiguous block (tiny descriptor count, line rate).
    # xq[p, q, c, t] = x[t=q*512+t, c*128+p]
    xq_d = nc.dram_tensor("xq", [128, NCH * CT * 512], BF16, kind="ExternalInput")
    # wqk[p, s, c, m, n] = w_attn[c*128+p, s*C + (4*s+m... see host prep
    wqk_d = nc.dram_tensor("wqk", [128, 2 * CT * CG], BF16, kind="ExternalInput")
    wv_d = nc.dram_tensor("wv", [128, CT * CG], BF16, kind="ExternalInput")
    wp_d = nc.dram_tensor("wp", [128, NP * C], BF16, kind="ExternalInput")
    bqkT_d = nc.dram_tensor("bqkT", [128, 8], F32, kind="ExternalInput")
    bv_d = nc.dram_tensor("bv", [1, CG], BF16, kind="ExternalInput")
    out_d = nc.dram_tensor("out", [T, C], BF16, kind="ExternalOutput")

    with tile.TileContext(nc) as tc, ExitStack() as ctx:
        pers = ctx.enter_context(tc.tile_pool(name="pers", bufs=1))

        # Head-pair qT/kT tiles in [d, t] layout: head 2m at partitions
        # 0-63, head 2m+1 at partitions 64-127.
        qTp = [pers.tile([128, T], BF16, name=f"qTp{m}") for m in range(NP)]
        kTp = [pers.tile([128, T], BF16, name=f"kTp{m}") for m in range(NP)]
        # v_aug[p, j, h, 0:64] = 1.0; [..., 64:128] = v[t=j*128+p, h*64+d]
        # (64 replicated ones-columns make the AV matmul emit the softmax
        # denominator on partitions 0-63 — base-0 so reciprocal_approx_fast
        # can read it directly; the custom-DVE op breaks on partition-offset
        # APs).
        v_aug = pers.tile([128, TB, HG, 128], BF16, name="v_aug")
        utri = pers.tile([128, 128], BF16, name="utri")
        ones_q = pers.tile([1, 128], BF16, name="ones_q")
        bqkT_sb = pers.tile([128, 8], F32, name="bqkT_sb")
        bv_sb = pers.tile([1, CG], BF16, name="bv_sb")

        wqk_sb = pers.tile([128, 2, CT, 512], BF16, name="wqk_sb")
        wv_sb = pers.tile([128, CT, CG], BF16, name="wv_sb")
        wp_sb = pers.tile([128, NP, C], BF16, name="wp_sb")
        yT = [pers.tile([128, T], BF16, name=f"yT{m}") for m in range(NP)]

        # ---- input DMAs: contiguous blocks, two HWDGE rings ----
        xq_pool = ctx.enter_context(tc.tile_pool(name="xq_pool", bufs=2))
        xq_by_q = {}

        def p1_dma(q, split=1):
            xq = xq_pool.tile([128, CT, 512], BF16, name="xq", tag="xq")
            w = CT * 512
            for s in range(split):
                lo, hi = s * w // split, (s + 1) * w // split
                nc.sync.dma_start(
                    xq[:].rearrange("p c t -> p (c t)")[:, lo:hi],
                    xq_d.ap()[:, q * w + lo:q * w + hi])
            xq_by_q[q] = xq

        p1_dma(0, split=4)
        wvf = wv_sb[:].rearrange("p c n -> p (c n)")
        for s in range(4):
            nc.scalar.dma_start(wvf[:, s * 1024:(s + 1) * 1024],
                                wv_d.ap()[:, s * 1024:(s + 1) * 1024])
        nc.scalar.dma_start(bv_sb[:], bv_d.ap()[:])
        # wqk_sb[p, s, c, n]: s=0 q-part (all c contiguous), s=1 k-part
        wqkf = wqk_sb[:].rearrange("p s c n -> p (s c n)")
        nc.scalar.dma_start(wqkf[:, 0:4096], wqk_d.ap()[:, 0:4096])
        p1_dma(1)
        nc.scalar.dma_start(wqkf[:, 4096:8192], wqk_d.ap()[:, 4096:8192])
        nc.scalar.dma_start(bqkT_sb[:], bqkT_d.ap()[:])
        nc.scalar.dma_start(
            wp_sb[:].rearrange("p ct n -> p (ct n)"), wp_d.ap()[:])
        p1_dma(2)
        p1_dma(3)

        # Constants (after the DMAs so those issue first; emitted before
        # the v-copies / masks that share tiles with them).
        for j in range(TB):
            nc.vector.memset(v_aug[:, j, :, 0:64], 1.0)
        make_upper_triangular(nc, utri[:, :], val=1.0, diag=True)
        nc.vector.memset(ones_q[:], 1.0)

        # ---- PSUM pools ----
        sc_pool = ctx.enter_context(
            tc.tile_pool(name="sc_pool", bufs=2, space="PSUM"))   # 4 banks
        ps_y_pool = ctx.enter_context(
            tc.tile_pool(name="ps_y_pool", bufs=1, space="PSUM"))  # 2 banks
        pp_pool = ctx.enter_context(
            tc.tile_pool(name="pp_pool", bufs=2, space="PSUM"))   # 2 banks

        att_pool = ctx.enter_context(tc.tile_pool(name="att_pool", bufs=6))
        nrm_pool = ctx.enter_context(tc.tile_pool(name="nrm_pool", bufs=4))
        out_pool = ctx.enter_context(tc.tile_pool(name="out_pool", bufs=3))

        # ---- projection units ----
        def p1_v_unit(q, tb):
            """V projection for t-block tb of quarter q."""
            xq = xq_by_q[q]
            pv = pp_pool.tile([128, CG], F32, name="pv", tag="pp")
            for c in range(CT):
                nc.tensor.matmul(
                    pv[:], xq[:, c, tb * 128:(tb + 1) * 128], wv_sb[:, c, :],
                    start=(c == 0), stop=False)
            nc.tensor.matmul(
                pv[:], ones_q[:], bv_sb[:], start=False, stop=True)
            j = q * 4 + tb
            nc.vector.tensor_copy(
                v_aug[:, j, :, 64:128], pv[:].rearrange("p (h d) -> p h d", h=HG))

        def p1_qk_unit(q, m):
            """Q (m<4) or K (m>=4) projection block m of quarter q."""
            xq = xq_by_q[q]
            pqk = pp_pool.tile([128, 512], F32, name="pqk", tag="pp")
            s, mm = divmod(m, NP)
            for c in range(CT):
                nc.tensor.matmul(
                    pqk[:], wqk_sb[:, s, c, mm * 128:(mm + 1) * 128], xq[:, c, :],
                    start=(c == 0), stop=(c == CT - 1))
            dst = qTp[m] if m < NP else kTp[m - NP]
            sl = slice(q * 512, (q + 1) * 512)
            nc.vector.tensor_scalar_add(dst[:, sl], pqk[:], bqkT_sb[:, m:m + 1])

        def p1_units(q):
            for tb in range(4):
                yield lambda tb=tb: p1_v_unit(q, tb)
            for m in range(8):
                yield lambda m=m: p1_qk_unit(q, m)

        # ---- attention ----
        def attn_qk_exp(p, cch, j):
            """QK row-tiled pair -> exp -> mask for one (pair, chunk, j)."""
            dead = max(0, j - 4 * cch) * 128
            q0 = cch * 512
            ps = sc_pool.tile([128, 1024], F32, name="ps", tag="sc")
            nc.tensor.matmul(
                ps[:, dead:512],
                kTp[p][0:64, j * 128:(j + 1) * 128],
                qTp[p][0:64, q0 + dead:q0 + 512],
                start=True, stop=True)
            nc.tensor.matmul(
                ps[:, 512 + dead:1024],
                kTp[p][64:128, j * 128:(j + 1) * 128],
                qTp[p][64:128, q0 + dead:q0 + 512],
                start=True, stop=True)
            att = att_pool.tile([128, 1024], BF16, tag="att")
            nc.scalar.activation(
                att[:, dead:1024], ps[:, dead:1024], EXP, scale=0.125)
            if j >= 4 * cch:
                nc.vector.tensor_tensor(
                    out=att[:, dead:dead + 128], in0=att[:, dead:dead + 128],
                    in1=utri[:, :], op=MUL)
                nc.vector.tensor_tensor(
                    out=att[:, 512 + dead:640 + dead],
                    in0=att[:, 512 + dead:640 + dead],
                    in1=utri[:, :], op=MUL)
            return att, dead

        def attn_av(p, j, att, dead, ps_y2, first, last):
            nc.tensor.matmul(
                ps_y2[0][:, dead:512], v_aug[:, j, 2 * p, :],
                att[:, dead:512], start=first, stop=last)
            nc.tensor.matmul(
                ps_y2[1][:, dead:512], v_aug[:, j, 2 * p + 1, :],
                att[:, 512 + dead:1024], start=first, stop=last)

        def normalize(p, cch, ps_y2):
            sl = slice(cch * 512, (cch + 1) * 512)
            for hh in range(2):
                inv = nrm_pool.tile([64, 512], F32, tag="inv")
                nc.vector.reciprocal_approx_fast(inv[:], ps_y2[hh][0:64, :])
                rows = slice(0, 64) if hh == 0 else slice(64, 128)
                nc.vector.tensor_tensor(
                    out=yT[p][rows, sl], in0=ps_y2[hh][64:128, :],
                    in1=inv[:], op=MUL)

        def attn_chunk_pair(p, cch, interleave):
            """All j-steps of (pair, chunk); AV runs one step behind QK/exp
            so the PE never head-of-line blocks on the current exp."""
            ps_y2 = [ps_y_pool.tile([128, 512], F32, name=f"ps_y{hh}",
                                    tag=f"ps_y{hh}") for hh in range(2)]
            jmax = 4 * cch + 3
            pend = None
            for j in range(jmax + 1):
                att, dead = attn_qk_exp(p, cch, j)
                if pend is not None:
                    attn_av(p, *pend, ps_y2, first=(pend[0] == 0), last=False)
                pend = (j, att, dead)
                interleave()
            attn_av(p, *pend, ps_y2, first=(pend[0] == 0), last=True)
            normalize(p, cch, ps_y2)

        # ---- output projection ----
        def proj_ch(tb, ch, o_sb, cts=range(NP), pp=None, finish=True):
            if pp is None:
                pp = pp_pool.tile([128, 512], F32, name="pp", tag="pp")
            for ct in cts:
                nc.tensor.matmul(
                    pp[:],
                    yT[ct][:, tb * 128:(tb + 1) * 128],
                    wp_sb[:, ct, ch * 512:(ch + 1) * 512],
                    start=(ct == 0), stop=(finish and ct == NP - 1))
            if not finish:
                return pp
            nc.vector.tensor_copy(o_sb[:, ch * 512:(ch + 1) * 512], pp[:])
            nc.sync.dma_start(
                out_d.ap()[tb * 128:(tb + 1) * 128, ch * 512:(ch + 1) * 512],
                o_sb[:, ch * 512:(ch + 1) * 512])

        def proj_unit(tb):
            o_sb = out_pool.tile([128, C], BF16, tag="o_sb")
            for ch in range(2):
                proj_ch(tb, ch, o_sb)

        # ---------------- Orchestration ----------------
        # Quarter 0 straight through; attention chunk c needs Q quarter c
        # and K/V quarters <= c, so chunk c overlaps projection quarter
        # c+1 and the output projection of earlier chunks.
        for u in p1_units(0):
            u()

        fill_by_chunk = {
            0: list(p1_units(1)),
            1: list(p1_units(2)) + [lambda tb=tb: proj_unit(tb)
                                    for tb in range(0, 4)],
            2: list(p1_units(3)),
            3: [lambda tb=tb: proj_unit(tb) for tb in range(4, 12)],
        }

        for cch in range(NCH):
            fill = fill_by_chunk[cch]
            nsteps = NP * (4 * cch + 4)
            quota = [0] * nsteps
            for i in range(len(fill)):
                quota[(i * nsteps) // len(fill)] += 1
            step_i = [0]

            def interleave(quota=quota, step_i=step_i, fill=fill,
                           nsteps=nsteps):
                i = step_i[0]
                step_i[0] += 1
                for _ in range(quota[i] if i < nsteps else 0):
                    fill.pop(0)()

            for p in range(NP):
                attn_chunk_pair(p, cch, interleave)
            for u in fill:
                u()

        # Tail: tb12/13's ct0-2 partials are independent of pair 3 and fill
        # the last-normalize latency; ct3 finishes each open group after.
        for tb in (12, 13):
            o_sb = out_pool.tile([128, C], BF16, tag="o_sb")
            pps = [proj_ch(tb, ch, o_sb, cts=range(3), finish=False)
                   for ch in range(2)]
            for ch in range(2):
                proj_ch(tb, ch, o_sb, cts=[3], pp=pps[ch])
        for tb in (14, 15):
            proj_unit(tb)

    nc.compile()
    return nc


def _get_nc():
    if "nc" not in _NC_CACHE:
        _register_ntff_hook()
        _NC_CACHE["nc"] = _build()
    return _NC_CACHE["nc"]


def kernel(x, w_attn, b_attn, w_proj, b_proj, _run_kwargs=None):
    import ml_dtypes
    from concourse.bass_utils import run_bass_kernel_spmd

    bf16 = ml_dtypes.bfloat16
    x = np.asarray(x, dtype=np.float32)
    w_attn = np.asarray(w_attn, dtype=np.float32)
    b_attn = np.asarray(b_attn, dtype=np.float32)
    w_proj = np.asarray(w_proj, dtype=np.float32)
    b_proj = np.asarray(b_proj, dtype=np.float32)

    nc = _get_nc()
    in_maps = []
    for core in range(NCORES):
        b, g = divmod(core, 2)
        cols = slice(g * CG, (g + 1) * CG)
        # xq[p, q, c, t] = x[b, q*512+t, c*128+p]
        xq = np.ascontiguousarray(
            x[b].reshape(NCH, 512, CT, 128).transpose(3, 0, 2, 1)
        ).reshape(128, -1)
        # wqk[p, s, c, n] = w_attn[c*128+p, s*C + g*CG + n]
        wqk = np.stack(
            [w_attn[:, cols], w_attn[:, C + g * CG: C + (g + 1) * CG]],
            axis=0).reshape(2, CT, 128, CG).transpose(2, 0, 1, 3)
        wv = w_attn[:, 2 * C + g * CG: 2 * C + (g + 1) * CG]
        bqk = np.concatenate(
            [b_attn[cols], b_attn[C + g * CG: C + (g + 1) * CG]])
        in_maps.append({
            "xq": xq.astype(bf16),
            "wqk": np.ascontiguousarray(wqk).reshape(128, -1).astype(bf16),
            "wv": np.ascontiguousarray(
                wv.reshape(CT, 128, CG).transpose(1, 0, 2)
            ).reshape(128, -1).astype(bf16),
            "wp": np.ascontiguousarray(
                w_proj[g * CG:(g + 1) * CG, :].reshape(NP, 128, C)
                .transpose(1, 0, 2)).reshape(128, -1).astype(bf16),
            "bqkT": np.ascontiguousarray(
                bqk.reshape(8, 128).T).astype(np.float32),
            "bv": np.ascontiguousarray(
                b_attn[2 * C + g * CG: 2 * C + (g + 1) * CG]).reshape(1, -1).astype(bf16),
        })

    res = run_bass_kernel_spmd(nc, in_maps, core_ids=list(range(NCORES)),
                               **(_run_kwargs or {}))
    out = np.empty((B, T, C), dtype=np.float32)
    for b in range(B):
        out[b] = (res.results[2 * b]["out"].astype(np.float32)
                  + res.results[2 * b + 1]["out"].astype(np.float32) + b_proj)
    if _run_kwargs:
        kernel.last_results = res
    return out


# revision 46
# speedup vs baseline: 1.0064x; 1.0064x over previous
"""Causal self-attention (B=4, T=2048, C=1024, H=16, D=64) on 8 TRN2 NeuronCores.

Sharding: 8 cores = 4 batches x 2 head-groups (8 heads each). Each core:
  - QKV projection for its (batch, head-group) column slice of w_attn,
    producing qT/kT in [d, t] layout and v in [t, d].
  - Causal attention with softmax denominators from 64 replicated
    ones-columns appended to V (no cross-partition reduction needed).
  - Row-sharded output projection -> per-core partial [T, C].
Host sums the two partials per batch and adds b_proj.

Head-pair layout: heads (2m, 2m+1) share one [128, T] qT/kT tile
(partitions 0-63 / 64-127), so the two K=64 QK matmuls of a pair run
concurrently as row-tiles of the PE array (tile_position (0,0)/(64,0)),
writing the two 512-col halves of one [128, 1024] PSUM tile.  A single
exp activation then covers both heads, halving ACT instruction count.
Attention is processed in 512-wide q-chunks; chunk c only needs
projection quarter c of Q (and quarters <= c of K/V), so attention
starts right after quarter 0 and the projection quarters + the output
projection interleave with it to keep the PE busy while ACT runs exp.

Matmul operands are bf16 (1 cycle/row on the PE) with all accumulation
in fp32 PSUM.  Input DMAs are batched into ~10 large descriptors spread
over the SP and ACT HWDGE rings.
"""

import sys
import types

import numpy as np

B, T, C, H, D = 4, 2048, 1024, 16, 64
HG = 8            # heads per core
CG = HG * D       # 512 channels per group
NP = HG // 2      # 4 head-pairs per core
NCORES = 8
TB = T // 128     # 16 t-blocks (also k-blocks)
CT = C // 128     # 8 c-chunks of the contraction dim
NCH = 4           # 512-wide q-chunks


def _register_ntff_hook():
    """Register the axon NTFF profile hook if the image's antenv lacks it."""
    try:
        import antenv
        if getattr(antenv, "axon_hooks", None) is not None:
            return
        from trn_agent_boot.trn_boot import _ntff_profile_via_ctypes
        hook = _ntff_profile_via_ctypes("/opt/axon/libaxon_pjrt.so")
        mod = types.ModuleType("antenv.axon_hooks")
        mod._hook = hook
        mod.get_axon_ntff_profile_hook = lambda: mod._hook
        mod.set_axon_ntff_profile_hook = lambda h: setattr(mod, "_hook", h)
        sys.modules["antenv.axon_hooks"] = mod
        antenv.axon_hooks = mod
    except Exception:
        pass


_NC_CACHE = {}


def _build():
    import concourse.bacc as bacc
    import concourse.mybir as mybir
    import concourse.tile as tile
    from concourse.masks import make_upper_triangular
    from contextlib import ExitStack

    F32 = mybir.dt.float32
    BF16 = mybir.dt.bfloat16
    MUL = mybir.AluOpType.mult
    EXP = mybir.ActivationFunctionType.Exp

    nc = bacc.Bacc(None, target_bir_lowering=False, debug=False)
    # All inputs host-pre-arranged to [128, ...] partition-major layouts so
    # every DMA is a contiguous block (tiny descriptor count, line rate).
    # xq[p, q, c, t] = x[t=q*512+t, c*128+p]
    xq_d = nc.dram_tensor("xq", [128, NCH * CT * 512], BF16, kind="ExternalInput")
    # wqk[p, s, c, m, n] = w_attn[c*128+p, s*C + (4*s+m... see host prep
    wqk_d = nc.dram_tensor("wqk", [128, 2 * CT * CG], BF16, kind="ExternalInput")
    wv_d = nc.dram_tensor("wv", [128, CT * CG], BF16, kind="ExternalInput")
    wp_d = nc.dram_tensor("wp", [128, NP * C], BF16, kind="ExternalInput")
    bqkT_d = nc.dram_tensor("bqkT", [128, 8], F32, kind="ExternalInput")
    bv_d = nc.dram_tensor("bv", [1, CG], BF16, kind="ExternalInput")
    out_d = nc.dram_tensor("out", [T, C], BF16, kind="ExternalOutput")

    with tile.TileContext(nc) as tc, ExitStack() as ctx:
        pers = ctx.enter_context(tc.tile_pool(name="pers", bufs=1))

        # Head-pair qT/kT tiles in [d, t] layout: head 2m at partitions
        # 0-63, head 2m+1 at partitions 64-127.
        qTp = [pers.tile([128, T], BF16, name=f"qTp{m}") for m in range(NP)]
        kTp = [pers.tile([128, T], BF16, name=f"kTp{m}") for m in range(NP)]
        # v_aug[p, j, h, 0:64] = 1.0; [..., 64:128] = v[t=j*128+p, h*64+d]
        # (64 replicated ones-columns make the AV matmul emit the softmax
        # denominator on partitions 0-63 — base-0 so reciprocal_approx_fast
        # can read it directly; the custom-DVE op breaks on partition-offset
        # APs).
        v_aug = pers.tile([128, TB, HG, 128], BF16, name="v_aug")
        utri = pers.tile([128, 128], BF16, name="utri")
        ones_q = pers.tile([1, 128], BF16, name="ones_q")
        bqkT_sb = pers.tile([128, 8], F32, name="bqkT_sb")
        bv_sb = pers.tile([1, CG], BF16, name="bv_sb")

        wqk_sb = pers.tile([128, 2, CT, 512], BF16, name="wqk_sb")
        wv_sb = pers.tile([128, CT, CG], BF16, name="wv_sb")
        wp_sb = pers.tile([128, NP, C], BF16, name="wp_sb")
        yT = [pers.tile([128, T], BF16, name=f"yT{m}") for m in range(NP)]

        # ---- input DMAs: contiguous blocks, two HWDGE rings ----
        xq_pool = ctx.enter_context(tc.tile_pool(name="xq_pool", bufs=2))
        xq_by_q = {}

        def p1_dma(q, split=1):
            xq = xq_pool.tile([128, CT, 512], BF16, name="xq", tag="xq")
            w = CT * 512
            for s in range(split):
                lo, hi = s * w // split, (s + 1) * w // split
                nc.sync.dma_start(
                    xq[:].rearrange("p c t -> p (c t)")[:, lo:hi],
                    xq_d.ap()[:, q * w + lo:q * w + hi])
            xq_by_q[q] = xq

        p1_dma(0, split=4)
        wvf = wv_sb[:].rearrange("p c n -> p (c n)")
        for s in range(4):
            nc.scalar.dma_start(wvf[:, s * 1024:(s + 1) * 1024],
                                wv_d.ap()[:, s * 1024:(s + 1) * 1024])
        nc.scalar.dma_start(bv_sb[:], bv_d.ap()[:])
        # wqk_sb[p, s, c, n]: s=0 q-part (all c contiguous), s=1 k-part
        wqkf = wqk_sb[:].rearrange("p s c n -> p (s c n)")
        nc.scalar.dma_start(wqkf[:, 0:4096], wqk_d.ap()[:, 0:4096])
        p1_dma(1)
        nc.scalar.dma_start(wqkf[:, 4096:8192], wqk_d.ap()[:, 4096:8192])
        nc.scalar.dma_start(bqkT_sb[:], bqkT_d.ap()[:])
        nc.scalar.dma_start(
            wp_sb[:].rearrange("p ct n -> p (ct n)"), wp_d.ap()[:])
        p1_dma(2)
        p1_dma(3)

        # Constants (after the DMAs so those issue first; emitted before
        # the v-copies / masks that share tiles with them).
        for j in range(TB):
            nc.vector.memset(v_aug[:, j, :, 0:64], 1.0)
        make_upper_triangular(nc, utri[:, :], val=1.0, diag=True)
        nc.vector.memset(ones_q[:], 1.0)

        # ---- PSUM pools ----
        sc_pool = ctx.enter_context(
            tc.tile_pool(name="sc_pool", bufs=2, space="PSUM"))   # 4 banks
        ps_y_pool = ctx.enter_context(
            tc.tile_pool(name="ps_y_pool", bufs=1, space="PSUM"))  # 2 banks
        pp_pool = ctx.enter_context(
            tc.tile_pool(name="pp_pool", bufs=2, space="PSUM"))   # 2 banks

        att_pool = ctx.enter_context(tc.tile_pool(name="att_pool", bufs=6))
        nrm_pool = ctx.enter_context(tc.tile_pool(name="nrm_pool", bufs=4))
        out_pool = ctx.enter_context(tc.tile_pool(name="out_pool", bufs=3))

        # ---- projection units ----
        def p1_v_unit(q, tb):
            """V projection for t-block tb of quarter q."""
            xq = xq_by_q[q]
            pv = pp_pool.tile([128, CG], F32, name="pv", tag="pp")
            for c in range(CT):
                nc.tensor.matmul(
                    pv[:], xq[:, c, tb * 128:(tb + 1) * 128], wv_sb[:, c, :],
                    start=(c == 0), stop=False)
            nc.tensor.matmul(
                pv[:], ones_q[:], bv_sb[:], start=False, stop=True)
            j = q * 4 + tb
            nc.vector.tensor_copy(
                v_aug[:, j, :, 64:128], pv[:].rearrange("p (h d) -> p h d", h=HG))

        def p1_qk_unit(q, m):
            """Q (m<4) or K (m>=4) projection block m of quarter q."""
            xq = xq_by_q[q]
            pqk = pp_pool.tile([128, 512], F32, name="pqk", tag="pp")
            s, mm = divmod(m, NP)
            for c in range(CT):
                nc.tensor.matmul(
                    pqk[:], wqk_sb[:, s, c, mm * 128:(mm + 1) * 128], xq[:, c, :],
                    start=(c == 0), stop=(c == CT - 1))
            dst = qTp[m] if m < NP else kTp[m - NP]
            sl = slice(q * 512, (q + 1) * 512)
            nc.vector.tensor_scalar_add(dst[:, sl], pqk[:], bqkT_sb[:, m:m + 1])

        def p1_units(q):
            for tb in range(4):
                yield lambda tb=tb: p1_v_unit(q, tb)
            for m in range(8):
                yield lambda m=m: p1_qk_unit(q, m)

        # ---- attention ----
        def attn_qk_exp(p, cch, j):
            """QK row-tiled pair -> exp -> mask for one (pair, chunk, j)."""
            dead = max(0, j - 4 * cch) * 128
            q0 = cch * 512
            ps = sc_pool.tile([128, 1024], F32, name="ps", tag="sc")
            nc.tensor.matmul(
                ps[:, dead:512],
                kTp[p][0:64, j * 128:(j + 1) * 128],
                qTp[p][0:64, q0 + dead:q0 + 512],
                start=True, stop=True)
            nc.tensor.matmul(
                ps[:, 512 + dead:1024],
                kTp[p][64:128, j * 128:(j + 1) * 128],
                qTp[p][64:128, q0 + dead:q0 + 512],
                start=True, stop=True)
            att = att_pool.tile([128, 1024], BF16, tag="att")
            nc.scalar.activation(
                att[:, dead:1024], ps[:, dead:1024], EXP, scale=0.125)
            if j >= 4 * cch:
                nc.vector.tensor_tensor(
                    out=att[:, dead:dead + 128], in0=att[:, dead:dead + 128],
                    in1=utri[:, :], op=MUL)
                nc.vector.tensor_tensor(
                    out=att[:, 512 + dead:640 + dead],
                    in0=att[:, 512 + dead:640 + dead],
                    in1=utri[:, :], op=MUL)
            return att, dead

        def attn_av(p, j, att, dead, ps_y2, first, last):
            nc.tensor.matmul(
                ps_y2[0][:, dead:512], v_aug[:, j, 2 * p, :],
                att[:, dead:512], start=first, stop=last)
            nc.tensor.matmul(
                ps_y2[1][:, dead:512], v_aug[:, j, 2 * p + 1, :],
                att[:, 512 + dead:1024], start=first, stop=last)

        def normalize(p, cch, ps_y2):
            sl = slice(cch * 512, (cch + 1) * 512)
            for hh in range(2):
                inv = nrm_pool.tile([64, 512], F32, tag="inv")
                nc.vector.reciprocal_approx_fast(inv[:], ps_y2[hh][0:64, :])
                rows = slice(0, 64) if hh == 0 else slice(64, 128)
                nc.vector.tensor_tensor(
                    out=yT[p][rows, sl], in0=ps_y2[hh][64:128, :],
                    in1=inv[:], op=MUL)

        def attn_chunk_pair(p, cch, interleave):
            """All j-steps of (pair, chunk); AV runs one step behind QK/exp
            so the PE never head-of-line blocks on the current exp."""
            ps_y2 = [ps_y_pool.tile([128, 512], F32, name=f"ps_y{hh}",
                                    tag=f"ps_y{hh}") for hh in range(2)]
            jmax = 4 * cch + 3
            pend = None
            for j in range(jmax + 1):
                att, dead = attn_qk_exp(p, cch, j)
                if pend is not None:
                    attn_av(p, *pend, ps_y2, first=(pend[0] == 0), last=False)
                pend = (j, att, dead)
                interleave()
            attn_av(p, *pend, ps_y2, first=(pend[0] == 0), last=True)
            normalize(p, cch, ps_y2)

        # ---- output projection ----
        def proj_ch(tb, ch, o_sb, cts=range(NP), pp=None, finish=True):
            if pp is None:
                pp = pp_pool.tile([128, 512], F32, name="pp", tag="pp")
            for ct in cts:
                nc.tensor.matmul(
                    pp[:],
                    yT[ct][:, tb * 128:(tb + 1) * 128],
                    wp_sb[:, ct, ch * 512:(ch + 1) * 512],
                    start=(ct == 0), stop=(finish and ct == NP - 1))
            if not finish:
                return pp
            nc.vector.tensor_copy(o_sb[:, ch * 512:(ch + 1) * 512], pp[:])
            if ch == 1:
                nc.sync.dma_start(
                    out_d.ap()[tb * 128:(tb + 1) * 128, :], o_sb[:])

        def proj_unit(tb):
            o_sb = out_pool.tile([128, C], BF16, tag="o_sb")
            for ch in range(2):
                proj_ch(tb, ch, o_sb)

        # ---------------- Orchestration ----------------
        # Quarter 0 straight through; attention chunk c needs Q quarter c
        # and K/V quarters <= c, so chunk c overlaps projection quarter
        # c+1 and the output projection of earlier chunks.
        for u in p1_units(0):
            u()

        fill_by_chunk = {
            0: list(p1_units(1)),
            1: list(p1_units(2)) + [lambda tb=tb: proj_unit(tb)
                                    for tb in range(0, 4)],
            2: list(p1_units(3)),
            3: [lambda tb=tb: proj_unit(tb) for tb in range(4, 12)],
        }

        for cch in range(NCH):
            fill = fill_by_chunk[cch]
            nsteps = NP * (4 * cch + 4)
            quota = [0] * nsteps
            for i in range(len(fill)):
                quota[(i * nsteps) // len(fill)] += 1
            step_i = [0]

            def interleave(quota=quota, step_i=step_i, fill=fill,
                           nsteps=nsteps):
                i = step_i[0]
                step_i[0] += 1
                for _ in range(quota[i] if i < nsteps else 0):
                    fill.pop(0)()

            for p in range(NP):
                attn_chunk_pair(p, cch, interleave)
            for u in fill:
                u()

        # Tail: tb12/13's ct0-2 partials are independent of pair 3 and fill
        # the last-normalize latency; ct3 finishes each open group after.
        for tb in (12, 13):
            o_sb = out_pool.tile([128, C], BF16, tag="o_sb")
            pps = [proj_ch(tb, ch, o_sb, cts=range(3), finish=False)
                   for ch in range(2)]
            for ch in range(2):
                proj_ch(tb, ch, o_sb, cts=[3], pp=pps[ch])
        for tb in (14, 15):
            proj_unit(tb)

    nc.compile()
    return nc


def _get_nc():
    if "nc" not in _NC_CACHE:
        _register_ntff_hook()
        _NC_CACHE["nc"] = _build()
    return _NC_CACHE["nc"]


def kernel(x, w_attn, b_attn, w_proj, b_proj, _run_kwargs=None):
    import ml_dtypes
    from concourse.bass_utils import run_bass_kernel_spmd

    bf16 = ml_dtypes.bfloat16
    x = np.asarray(x, dtype=np.float32)
    w_attn = np.asarray(w_attn, dtype=np.float32)
    b_attn = np.asarray(b_attn, dtype=np.float32)
    w_proj = np.asarray(w_proj, dtype=np.float32)
    b_proj = np.asarray(b_proj, dtype=np.float32)

    nc = _get_nc()
    in_maps = []
    for core in range(NCORES):
        b, g = divmod(core, 2)
        cols = slice(g * CG, (g + 1) * CG)
        # xq[p, q, c, t] = x[b, q*512+t, c*128+p]
        xq = np.ascontiguousarray(
            x[b].reshape(NCH, 512, CT, 128).transpose(3, 0, 2, 1)
        ).reshape(128, -1)
        # wqk[p, s, c, n] = w_attn[c*128+p, s*C + g*CG + n]
        wqk = np.stack(
            [w_attn[:, cols], w_attn[:, C + g * CG: C + (g + 1) * CG]],
            axis=0).reshape(2, CT, 128, CG).transpose(2, 0, 1, 3)
        wv = w_attn[:, 2 * C + g * CG: 2 * C + (g + 1) * CG]
        bqk = np.concatenate(
            [b_attn[cols], b_attn[C + g * CG: C + (g + 1) * CG]])
        in_maps.append({
            "xq": xq.astype(bf16),
            "wqk": np.ascontiguousarray(wqk).reshape(128, -1).astype(bf16),
            "wv": np.ascontiguousarray(
                wv.reshape(CT, 128, CG).transpose(1, 0, 2)
            ).reshape(128, -1).astype(bf16),
            "wp": np.ascontiguousarray(
                w_proj[g * CG:(g + 1) * CG, :].reshape(NP, 128, C)
                .transpose(1, 0, 2)).reshape(128, -1).astype(bf16),
            "bqkT": np.ascontiguousarray(
                bqk.reshape(8, 128).T).astype(np.float32),
            "bv": np.ascontiguousarray(
                b_attn[2 * C + g * CG: 2 * C + (g + 1) * CG]).reshape(1, -1).astype(bf16),
        })

    res = run_bass_kernel_spmd(nc, in_maps, core_ids=list(range(NCORES)),
                               **(_run_kwargs or {}))
    out = np.empty((B, T, C), dtype=np.float32)
    for b in range(B):
        out[b] = (res.results[2 * b]["out"].astype(np.float32)
                  + res.results[2 * b + 1]["out"].astype(np.float32) + b_proj)
    if _run_kwargs:
        kernel.last_results = res
    return out


# revision 47
# speedup vs baseline: 1.0424x; 1.0357x over previous
"""Causal self-attention (B=4, T=2048, C=1024, H=16, D=64) on 8 TRN2 NeuronCores.

Sharding: 8 cores = 4 batches x 2 head-groups (8 heads each). Each core:
  - QKV projection for its (batch, head-group) column slice of w_attn,
    producing qT/kT in [d, t] layout and v in [t, d].
  - Causal attention with softmax denominators from 64 replicated
    ones-columns appended to V (no cross-partition reduction needed).
  - Row-sharded output projection -> per-core partial [T, C].
Host sums the two partials per batch and adds b_proj.

Head-pair layout: heads (2m, 2m+1) share one [128, T] qT/kT tile
(partitions 0-63 / 64-127), so the two K=64 QK matmuls of a pair run
concurrently as row-tiles of the PE array (tile_position (0,0)/(64,0)),
writing the two 512-col halves of one [128, 1024] PSUM tile.  A single
exp activation then covers both heads, halving ACT instruction count.
Attention is processed in 512-wide q-chunks; chunk c only needs
projection quarter c of Q (and quarters <= c of K/V), so attention
starts right after quarter 0 and the projection quarters + the output
projection interleave with it to keep the PE busy while ACT runs exp.

Matmul operands are bf16 (1 cycle/row on the PE) with all accumulation
in fp32 PSUM.  Input DMAs are batched into ~10 large descriptors spread
over the SP and ACT HWDGE rings.
"""

import sys
import types

import numpy as np

B, T, C, H, D = 4, 2048, 1024, 16, 64
HG = 8            # heads per core
CG = HG * D       # 512 channels per group
NP = HG // 2      # 4 head-pairs per core
NCORES = 8
TB = T // 128     # 16 t-blocks (also k-blocks)
CT = C // 128     # 8 c-chunks of the contraction dim
NCH = 4           # 512-wide q-chunks


def _register_ntff_hook():
    """Register the axon NTFF profile hook if the image's antenv lacks it."""
    try:
        import antenv
        if getattr(antenv, "axon_hooks", None) is not None:
            return
        from trn_agent_boot.trn_boot import _ntff_profile_via_ctypes
        hook = _ntff_profile_via_ctypes("/opt/axon/libaxon_pjrt.so")
        mod = types.ModuleType("antenv.axon_hooks")
        mod._hook = hook
        mod.get_axon_ntff_profile_hook = lambda: mod._hook
        mod.set_axon_ntff_profile_hook = lambda h: setattr(mod, "_hook", h)
        sys.modules["antenv.axon_hooks"] = mod
        antenv.axon_hooks = mod
    except Exception:
        pass


_NC_CACHE = {}


def _build():
    import concourse.bacc as bacc
    import concourse.mybir as mybir
    import concourse.tile as tile
    from concourse.masks import make_upper_triangular
    from contextlib import ExitStack

    F32 = mybir.dt.float32
    BF16 = mybir.dt.bfloat16
    MUL = mybir.AluOpType.mult
    EXP = mybir.ActivationFunctionType.Exp

    nc = bacc.Bacc(None, target_bir_lowering=False, debug=False)
    # All inputs host-pre-arranged to [128, ...] partition-major layouts so
    # every DMA is a contiguous block (tiny descriptor count, line rate).
    # xq[p, q, c, t] = x[t=q*512+t, c*128+p]
    xq_d = nc.dram_tensor("xq", [128, NCH * CT * 512], BF16, kind="ExternalInput")
    # wqk[p, s, c, m, n] = w_attn[c*128+p, s*C + (4*s+m... see host prep
    wqk_d = nc.dram_tensor("wqk", [128, 2 * CT * CG], BF16, kind="ExternalInput")
    wv_d = nc.dram_tensor("wv", [128, CT * CG], BF16, kind="ExternalInput")
    wp_d = nc.dram_tensor("wp", [128, NP * C], BF16, kind="ExternalInput")
    bqkT_d = nc.dram_tensor("bqkT", [128, 8], F32, kind="ExternalInput")
    bv_d = nc.dram_tensor("bv", [1, CG], BF16, kind="ExternalInput")
    out_d = nc.dram_tensor("out", [T, C], BF16, kind="ExternalOutput")

    with tile.TileContext(nc) as tc, ExitStack() as ctx:
        pers = ctx.enter_context(tc.tile_pool(name="pers", bufs=1))

        # Head-pair qT/kT tiles in [d, t] layout: head 2m at partitions
        # 0-63, head 2m+1 at partitions 64-127.
        qTp = [pers.tile([128, T], BF16, name=f"qTp{m}") for m in range(NP)]
        kTp = [pers.tile([128, T], BF16, name=f"kTp{m}") for m in range(NP)]
        # v_aug[p, j, h, 0:64] = 1.0; [..., 64:128] = v[t=j*128+p, h*64+d]
        # (64 replicated ones-columns make the AV matmul emit the softmax
        # denominator on partitions 0-63 — base-0 so reciprocal_approx_fast
        # can read it directly; the custom-DVE op breaks on partition-offset
        # APs).
        v_aug = pers.tile([128, TB, HG, 128], BF16, name="v_aug")
        utri = pers.tile([128, 128], BF16, name="utri")
        ones_q = pers.tile([1, 128], BF16, name="ones_q")
        bqkT_sb = pers.tile([128, 8], F32, name="bqkT_sb")
        bv_sb = pers.tile([1, CG], BF16, name="bv_sb")

        wqk_sb = pers.tile([128, 2, CT, 512], BF16, name="wqk_sb")
        wv_sb = pers.tile([128, CT, CG], BF16, name="wv_sb")
        wp_sb = pers.tile([128, NP, C], BF16, name="wp_sb")
        yT = [pers.tile([128, T], BF16, name=f"yT{m}") for m in range(NP)]

        # ---- input DMAs: contiguous blocks, two HWDGE rings ----
        xq_pool = ctx.enter_context(tc.tile_pool(name="xq_pool", bufs=2))
        xq_by_q = {}

        def p1_dma(q, split=1):
            xq = xq_pool.tile([128, CT, 512], BF16, name="xq", tag="xq")
            w = CT * 512
            for s in range(split):
                lo, hi = s * w // split, (s + 1) * w // split
                nc.sync.dma_start(
                    xq[:].rearrange("p c t -> p (c t)")[:, lo:hi],
                    xq_d.ap()[:, q * w + lo:q * w + hi])
            xq_by_q[q] = xq

        p1_dma(0, split=4)
        wvf = wv_sb[:].rearrange("p c n -> p (c n)")
        for s in range(4):
            nc.scalar.dma_start(wvf[:, s * 1024:(s + 1) * 1024],
                                wv_d.ap()[:, s * 1024:(s + 1) * 1024])
        nc.scalar.dma_start(bv_sb[:], bv_d.ap()[:])
        # wqk_sb[p, s, c, n]: s=0 q-part (all c contiguous), s=1 k-part
        wqkf = wqk_sb[:].rearrange("p s c n -> p (s c n)")
        nc.scalar.dma_start(wqkf[:, 0:4096], wqk_d.ap()[:, 0:4096])
        p1_dma(1)
        nc.scalar.dma_start(wqkf[:, 4096:8192], wqk_d.ap()[:, 4096:8192])
        nc.scalar.dma_start(bqkT_sb[:], bqkT_d.ap()[:])
        nc.scalar.dma_start(
            wp_sb[:].rearrange("p ct n -> p (ct n)"), wp_d.ap()[:])
        p1_dma(2)
        p1_dma(3)

        # Constants (after the DMAs so those issue first; emitted before
        # the v-copies / masks that share tiles with them).
        for j in range(TB):
            nc.vector.memset(v_aug[:, j, :, 0:64], 1.0)
        make_upper_triangular(nc, utri[:, :], val=1.0, diag=True)
        nc.vector.memset(ones_q[:], 1.0)

        # ---- PSUM pools ----
        sc_pool = ctx.enter_context(
            tc.tile_pool(name="sc_pool", bufs=2, space="PSUM"))   # 4 banks
        ps_y_pool = ctx.enter_context(
            tc.tile_pool(name="ps_y_pool", bufs=1, space="PSUM"))  # 2 banks
        pp_pool = ctx.enter_context(
            tc.tile_pool(name="pp_pool", bufs=2, space="PSUM"))   # 2 banks

        att_pool = ctx.enter_context(tc.tile_pool(name="att_pool", bufs=6))
        nrm_pool = ctx.enter_context(tc.tile_pool(name="nrm_pool", bufs=4))
        out_pool = ctx.enter_context(tc.tile_pool(name="out_pool", bufs=3))

        # ---- projection units ----
        def p1_v_unit(q, tb):
            """V projection for t-block tb of quarter q."""
            xq = xq_by_q[q]
            pv = pp_pool.tile([128, CG], F32, name="pv", tag="pp")
            for c in range(CT):
                nc.tensor.matmul(
                    pv[:], xq[:, c, tb * 128:(tb + 1) * 128], wv_sb[:, c, :],
                    start=(c == 0), stop=False)
            nc.tensor.matmul(
                pv[:], ones_q[:], bv_sb[:], start=False, stop=True)
            j = q * 4 + tb
            nc.vector.tensor_copy(
                v_aug[:, j, :, 64:128], pv[:].rearrange("p (h d) -> p h d", h=HG))

        def p1_qk_unit(q, m):
            """Q (m<4) or K (m>=4) projection block m of quarter q."""
            xq = xq_by_q[q]
            pqk = pp_pool.tile([128, 512], F32, name="pqk", tag="pp")
            s, mm = divmod(m, NP)
            for c in range(CT):
                nc.tensor.matmul(
                    pqk[:], wqk_sb[:, s, c, mm * 128:(mm + 1) * 128], xq[:, c, :],
                    start=(c == 0), stop=(c == CT - 1))
            dst = qTp[m] if m < NP else kTp[m - NP]
            sl = slice(q * 512, (q + 1) * 512)
            nc.vector.tensor_scalar_add(dst[:, sl], pqk[:], bqkT_sb[:, m:m + 1])

        def p1_units(q):
            for tb in range(4):
                yield lambda tb=tb: p1_v_unit(q, tb)
            for m in range(8):
                yield lambda m=m: p1_qk_unit(q, m)

        # ---- attention ----
        def attn_qk_exp(p, cch, j):
            """QK row-tiled pair -> exp -> mask for one (pair, chunk, j)."""
            dead = max(0, j - 4 * cch) * 128
            q0 = cch * 512
            ps = sc_pool.tile([128, 1024], F32, name="ps", tag="sc")
            nc.tensor.matmul(
                ps[:, dead:512],
                kTp[p][0:64, j * 128:(j + 1) * 128],
                qTp[p][0:64, q0 + dead:q0 + 512],
                start=True, stop=True)
            nc.tensor.matmul(
                ps[:, 512 + dead:1024],
                kTp[p][64:128, j * 128:(j + 1) * 128],
                qTp[p][64:128, q0 + dead:q0 + 512],
                start=True, stop=True)
            att = att_pool.tile([128, 1024], BF16, tag="att")
            nc.scalar.activation(
                att[:, dead:1024], ps[:, dead:1024], EXP, scale=0.125)
            if j >= 4 * cch:
                nc.vector.tensor_tensor(
                    out=att[:, dead:dead + 128], in0=att[:, dead:dead + 128],
                    in1=utri[:, :], op=MUL)
                nc.vector.tensor_tensor(
                    out=att[:, 512 + dead:640 + dead],
                    in0=att[:, 512 + dead:640 + dead],
                    in1=utri[:, :], op=MUL)
            return att, dead

        def attn_av(p, j, att, dead, ps_y2, first, last):
            nc.tensor.matmul(
                ps_y2[0][:, dead:512], v_aug[:, j, 2 * p, :],
                att[:, dead:512], start=first, stop=last)
            nc.tensor.matmul(
                ps_y2[1][:, dead:512], v_aug[:, j, 2 * p + 1, :],
                att[:, 512 + dead:1024], start=first, stop=last)

        def normalize(p, cch, ps_y2):
            sl = slice(cch * 512, (cch + 1) * 512)
            for hh in range(2):
                inv = nrm_pool.tile([64, 512], F32, tag="inv")
                nc.vector.reciprocal_approx_fast(inv[:], ps_y2[hh][0:64, :])
                rows = slice(0, 64) if hh == 0 else slice(64, 128)
                nc.vector.tensor_tensor(
                    out=yT[p][rows, sl], in0=ps_y2[hh][64:128, :],
                    in1=inv[:], op=MUL)

        def attn_chunk_pair(p, cch, interleave):
            """All j-steps of (pair, chunk), batched two at a time: the
            second QK pair's K=64 ldweights hides under the first pair's
            row-tiled matmuls, and the AVs of the previous batch form a
            clean full-row stream whose ldweights hide via the background
            weight buffer.  AV runs a batch behind QK/exp so the PE never
            head-of-line blocks on the current exp."""
            ps_y2 = [ps_y_pool.tile([128, 512], F32, name=f"ps_y{hh}",
                                    tag=f"ps_y{hh}") for hh in range(2)]
            jmax = 4 * cch + 3
            pend = []
            for j2 in range(0, jmax + 1, 2):
                a0 = attn_qk_exp(p, cch, j2)
                a1 = attn_qk_exp(p, cch, j2 + 1)
                for (jj, att, dead) in pend:
                    attn_av(p, jj, att, dead, ps_y2,
                            first=(jj == 0), last=False)
                pend = [(j2,) + a0, (j2 + 1,) + a1]
                interleave()
                interleave()
            for (jj, att, dead) in pend:
                attn_av(p, jj, att, dead, ps_y2,
                        first=(jj == 0), last=(jj == jmax))
            normalize(p, cch, ps_y2)

        # ---- output projection ----
        def proj_ch(tb, ch, o_sb, cts=range(NP), pp=None, finish=True):
            if pp is None:
                pp = pp_pool.tile([128, 512], F32, name="pp", tag="pp")
            for ct in cts:
                nc.tensor.matmul(
                    pp[:],
                    yT[ct][:, tb * 128:(tb + 1) * 128],
                    wp_sb[:, ct, ch * 512:(ch + 1) * 512],
                    start=(ct == 0), stop=(finish and ct == NP - 1))
            if not finish:
                return pp
            nc.vector.tensor_copy(o_sb[:, ch * 512:(ch + 1) * 512], pp[:])
            if ch == 1:
                nc.sync.dma_start(
                    out_d.ap()[tb * 128:(tb + 1) * 128, :], o_sb[:])

        def proj_unit(tb):
            o_sb = out_pool.tile([128, C], BF16, tag="o_sb")
            for ch in range(2):
                proj_ch(tb, ch, o_sb)

        # ---------------- Orchestration ----------------
        # Quarter 0 straight through; attention chunk c needs Q quarter c
        # and K/V quarters <= c, so chunk c overlaps projection quarter
        # c+1 and the output projection of earlier chunks.
        for u in p1_units(0):
            u()

        fill_by_chunk = {
            0: list(p1_units(1)),
            1: list(p1_units(2)) + [lambda tb=tb: proj_unit(tb)
                                    for tb in range(0, 4)],
            2: list(p1_units(3)),
            3: [lambda tb=tb: proj_unit(tb) for tb in range(4, 12)],
        }

        for cch in range(NCH):
            fill = fill_by_chunk[cch]
            nsteps = NP * (4 * cch + 4)
            quota = [0] * nsteps
            for i in range(len(fill)):
                quota[(i * nsteps) // len(fill)] += 1
            step_i = [0]

            def interleave(quota=quota, step_i=step_i, fill=fill,
                           nsteps=nsteps):
                i = step_i[0]
                step_i[0] += 1
                for _ in range(quota[i] if i < nsteps else 0):
                    fill.pop(0)()

            for p in range(NP):
                attn_chunk_pair(p, cch, interleave)
            for u in fill:
                u()

        # Tail: tb12/13's ct0-2 partials are independent of pair 3 and fill
        # the last-normalize latency; ct3 finishes each open group after.
        for tb in (12, 13):
            o_sb = out_pool.tile([128, C], BF16, tag="o_sb")
            pps = [proj_ch(tb, ch, o_sb, cts=range(3), finish=False)
                   for ch in range(2)]
            for ch in range(2):
                proj_ch(tb, ch, o_sb, cts=[3], pp=pps[ch])
        for tb in (14, 15):
            proj_unit(tb)

    nc.compile()
    return nc


def _get_nc():
    if "nc" not in _NC_CACHE:
        _register_ntff_hook()
        _NC_CACHE["nc"] = _build()
    return _NC_CACHE["nc"]


def kernel(x, w_attn, b_attn, w_proj, b_proj, _run_kwargs=None):
    import ml_dtypes
    from concourse.bass_utils import run_bass_kernel_spmd

    bf16 = ml_dtypes.bfloat16
    x = np.asarray(x, dtype=np.float32)
    w_attn = np.asarray(w_attn, dtype=np.float32)
    b_attn = np.asarray(b_attn, dtype=np.float32)
    w_proj = np.asarray(w_proj, dtype=np.float32)
    b_proj = np.asarray(b_proj, dtype=np.float32)

    nc = _get_nc()
    in_maps = []
    for core in range(NCORES):
        b, g = divmod(core, 2)
        cols = slice(g * CG, (g + 1) * CG)
        # xq[p, q, c, t] = x[b, q*512+t, c*128+p]
        xq = np.ascontiguousarray(
            x[b].reshape(NCH, 512, CT, 128).transpose(3, 0, 2, 1)
        ).reshape(128, -1)
        # wqk[p, s, c, n] = w_attn[c*128+p, s*C + g*CG + n]
        wqk = np.stack(
            [w_attn[:, cols], w_attn[:, C + g * CG: C + (g + 1) * CG]],
            axis=0).reshape(2, CT, 128, CG).transpose(2, 0, 1, 3)
        wv = w_attn[:, 2 * C + g * CG: 2 * C + (g + 1) * CG]
        bqk = np.concatenate(
            [b_attn[cols], b_attn[C + g * CG: C + (g + 1) * CG]])
        in_maps.append({
            "xq": xq.astype(bf16),
            "wqk": np.ascontiguousarray(wqk).reshape(128, -1).astype(bf16),
            "wv": np.ascontiguousarray(
                wv.reshape(CT, 128, CG).transpose(1, 0, 2)
            ).reshape(128, -1).astype(bf16),
            "wp": np.ascontiguousarray(
                w_proj[g * CG:(g + 1) * CG, :].reshape(NP, 128, C)
                .transpose(1, 0, 2)).reshape(128, -1).astype(bf16),
            "bqkT": np.ascontiguousarray(
                bqk.reshape(8, 128).T).astype(np.float32),
            "bv": np.ascontiguousarray(
                b_attn[2 * C + g * CG: 2 * C + (g + 1) * CG]).reshape(1, -1).astype(bf16),
        })

    res = run_bass_kernel_spmd(nc, in_maps, core_ids=list(range(NCORES)),
                               **(_run_kwargs or {}))
    out = np.empty((B, T, C), dtype=np.float32)
    for b in range(B):
        out[b] = (res.results[2 * b]["out"].astype(np.float32)
                  + res.results[2 * b + 1]["out"].astype(np.float32) + b_proj)
    if _run_kwargs:
        kernel.last_results = res
    return out
